# revision 1
# baseline (speedup 1.0000x reference)
"""Decode-step KV-cache attention kernel for 8 Trainium2 NeuronCores.

Strategy: tensor-parallel over heads (2 heads per core, all 32 batch rows on
every core) so the SPMD program is identical across cores; all per-core
differences live in the input data (host-sliced W_in columns, W_out rows and
head-slices of the caches).  Per batch row only the valid cache prefix
(input_pos tokens, rounded up to 128) is read from HBM - that is the memory
roofline for this problem.  The new-token k/v from the QKV projection are
folded in analytically (no cache scatter).  Softmax skips the max-subtraction
(scores are ~N(0,1) here; exp cannot overflow) and normalization is deferred:
PV accumulates unnormalized exp-weights, and the 1/sum scaling happens in the
single PSUM->SBUF copy at the end.

Output: each core produces attn_local @ W_out_rows(local heads) [32, 2048];
host sums the 8 partials and adds b_out.
"""

import math
import os
import sys

import numpy as np

sys.path.insert(0, "/opt/trn_rl_repo")

import concourse.bass as bass  # noqa: E402
import concourse.tile as tile  # noqa: E402
from concourse import bacc, mybir  # noqa: E402
from concourse.bass_utils import run_bass_kernel_spmd  # noqa: E402
from concourse.masks import make_identity  # noqa: E402

B, S_MAX, H, D = 32, 2048, 16, 128
E = H * D  # 2048
N_CORES = 8
H_LOC = H // N_CORES  # 2 heads per core
CLOC = H_LOC * D  # 256 channels per core
ST = 128  # sequence tile (partition dim)
ET = E // 128  # 16 contraction tiles for the in-projection

F32 = mybir.dt.float32
MULT = mybir.AluOpType.mult
ADD = mybir.AluOpType.add
EXP = mybir.ActivationFunctionType.Exp

_build_cache: dict = {}
LAST_RESULT = None  # last BassKernelResults, for test harness introspection

# bisect stages:
# 1 qkv only; 2 +newtoken ops; 3 +qdram store/qbcast DMAs; 4 +kv DMAs;
# 5 +scores; 6 +exp; 7 +PV matmuls; 99 full
STAGE = int(os.environ.get("KERNEL_STAGE", "99"))


def _build(n_ts: tuple, rems: tuple, stage: float = 99) -> bass.Bass:
    """Build the per-core Bass program (identical across cores)."""
    nc = bacc.Bacc("TRN2")
    x_d = nc.dram_tensor("x", [B, E], F32, kind="ExternalInput")
    win_d = nc.dram_tensor("win", [E, 3 * CLOC], F32, kind="ExternalInput")
    bin_d = nc.dram_tensor("bin", [1, 3 * CLOC], F32, kind="ExternalInput")
    wout_d = nc.dram_tensor("wout", [CLOC, E], F32, kind="ExternalInput")
    kc_d = nc.dram_tensor("kc", [B, S_MAX, CLOC], F32, kind="ExternalInput")
    vc_d = nc.dram_tensor("vc", [B, S_MAX, CLOC], F32, kind="ExternalInput")
    out_d = nc.dram_tensor("out", [B, E], F32, kind="ExternalOutput")
    q_dram = nc.dram_tensor("qscratch", [B, CLOC], F32, kind="Internal")

    inv_sqrt_d = 1.0 / math.sqrt(D)
    kc_ap = kc_d[:].rearrange("b (t p) c -> b p t c", p=128)
    vc_ap = vc_d[:].rearrange("b (t p) c -> b p t c", p=128)

    with tile.TileContext(nc) as tc:
        with tc.tile_pool(name="const", bufs=1) as const:
            I32 = const.tile([32, 32], F32)
            make_identity(nc, I32)
            ones_1x128 = const.tile([1, 128], F32)
            nc.vector.memset(ones_1x128, 1.0)
            ones_128 = const.tile([128, 1], F32)
            nc.vector.memset(ones_128, 1.0)
            ones_1x32 = const.tile([1, 32], F32)
            nc.vector.memset(ones_1x32, 1.0)
            ones_32 = const.tile([32, 1], F32)
            nc.vector.memset(ones_32, 1.0)

            win_sb = const.tile([128, ET, 3 * CLOC], F32)
            nc.sync.dma_start(
                out=win_sb, in_=win_d[:].rearrange("(t p) c -> p t c", p=128)
            )
            wout_sb = const.tile([128, H_LOC, E], F32)
            nc.sync.dma_start(
                out=wout_sb, in_=wout_d[:].rearrange("(t p) n -> p t n", p=128)
            )
            bin_sb = const.tile([1, 3 * CLOC], F32)
            nc.sync.dma_start(out=bin_sb, in_=bin_d[:])
            x_sb = const.tile([B, E], F32)
            nc.sync.dma_start(out=x_sb, in_=x_d[:])

            # unnormalized softmax partial sums per (head, batch) column
            sums_sb = const.tile([128, H_LOC * B], F32)
            nc.vector.memset(sums_sb, 0.0)

            q_sb = const.tile([B, CLOC], F32)
            k_new_sb = const.tile([B, CLOC], F32)
            v_new_sb = const.tile([B, CLOC], F32)
            snew_sb = const.tile([B, H_LOC], F32)
            e_new_sb = const.tile([B, H_LOC], F32)
            diag_sb = const.tile([32, H_LOC, 32], F32)
            xT_sb = const.tile([128, ET, B], F32)
            attn_sb = const.tile([128, H_LOC * B], F32)
            recip_sb = const.tile([1, H_LOC * B], F32)
            R_sb = const.tile([128, H_LOC * B], F32)
            out_sb = const.tile([B, E], F32)
            trash2 = const.tile([B, D], F32)

            # ---------------- phase 1: fused QKV projection ----------------
            with tc.tile_pool(name="ph1ps", bufs=2, space="PSUM") as ph1ps:
                with tc.tile_pool(name="qkvps", bufs=1, space="PSUM") as qkvps:
                    for t in range(ET):
                        xt_ps = ph1ps.tile([128, B], F32)
                        nc.tensor.transpose(
                            xt_ps, x_sb[:, t * 128 : (t + 1) * 128], I32
                        )
                        nc.vector.tensor_copy(xT_sb[:, t, :], xt_ps)
                    qkv_ps = qkvps.tile([B, 3 * CLOC], F32)
                    for c0, c1 in ((0, 512), (512, 768)):
                        # bias init (b_in broadcast via K=1 matmul), then accumulate
                        nc.tensor.matmul(
                            qkv_ps[:, c0:c1],
                            ones_1x32,
                            bin_sb[:, c0:c1],
                            start=True,
                            stop=False,
                        )
                        for t in range(ET):
                            nc.tensor.matmul(
                                qkv_ps[:, c0:c1],
                                xT_sb[:, t, :],
                                win_sb[:, t, c0:c1],
                                start=False,
                                stop=(t == ET - 1),
                            )
                    # q scaled by 1/sqrt(D) on the way out of PSUM
                    nc.scalar.mul(q_sb, qkv_ps[:, 0:CLOC], inv_sqrt_d)
                    nc.vector.tensor_copy(k_new_sb, qkv_ps[:, CLOC : 2 * CLOC])
                    nc.vector.tensor_copy(v_new_sb, qkv_ps[:, 2 * CLOC : 3 * CLOC])

            if stage >= 3:
                # park q in DRAM so it can be partition-broadcast by DMA below
                nc.sync.dma_start(out=q_dram[:], in_=q_sb)
            if stage >= 2:
                # new-token scores: e_new[b,h] = exp(q_bh . k_new_bh)
                for h in range(H_LOC):
                    nc.vector.tensor_mul(
                        trash2,
                        q_sb[:, h * D : (h + 1) * D],
                        k_new_sb[:, h * D : (h + 1) * D],
                    )
                    nc.vector.reduce_sum(
                        out=snew_sb[:, h : h + 1],
                        in_=trash2,
                        axis=mybir.AxisListType.X,
                    )
                if stage >= 2.2:
                    nc.scalar.activation(e_new_sb, snew_sb, EXP)
                if stage >= 2.4:
                    for h in range(H_LOC):
                        nc.vector.tensor_scalar_mul(
                            diag_sb[:, h, :], I32, e_new_sb[:, h : h + 1]
                        )

            # ---------------- phase 2: attention over cache prefixes --------
            with tc.tile_pool(name="attnps", bufs=1, space="PSUM") as attnps:
                attnT_ps = attnps.tile([128, H_LOC * B], F32)
                with tc.tile_pool(name="kv", bufs=2) as kvp, tc.tile_pool(
                    name="qb", bufs=3
                ) as qbp, tc.tile_pool(name="sc", bufs=3) as scp, tc.tile_pool(
                    name="pr", bufs=3
                ) as prp, tc.tile_pool(name="tr", bufs=4) as trp:
                    for b in range(B) if stage >= 3 else []:
                        n_t = n_ts[b]
                        rem = rems[b]
                        # broadcast q row b across 128 partitions via DMA
                        qrow = q_dram[b : b + 1, :]
                        qrow_bcast = bass.AP(
                            tensor=qrow.tensor,
                            offset=qrow.offset,
                            ap=[[0, 128]] + qrow.ap[1:],
                        )
                        qb_sb = qbp.tile([128, CLOC], F32)
                        nc.gpsimd.dma_start(out=qb_sb, in_=qrow_bcast)
                        if n_t == 0:
                            if stage >= 7:
                                for h in range(H_LOC):
                                    bh = h * B + b
                                    nc.tensor.matmul(
                                        attnT_ps[:, bh : bh + 1],
                                        v_new_sb[:, h * D : (h + 1) * D],
                                        diag_sb[:, h, b : b + 1],
                                        start=True,
                                        stop=True,
                                        skip_group_check=True,
                                    )
                            continue
                        if stage < 4:
                            continue
                        kt = kvp.tile([128, ET, CLOC], F32, tag="k")
                        vt = kvp.tile([128, ET, CLOC], F32, tag="v")
                        nc.sync.dma_start(
                            out=kt[:, :n_t, :], in_=kc_ap[b, :, 0:n_t, :]
                        )
                        nc.sync.dma_start(
                            out=vt[:, :n_t, :], in_=vc_ap[b, :, 0:n_t, :]
                        )
                        if stage < 5:
                            # consume the tiles so the DMAs aren't dead
                            nc.vector.tensor_copy(
                                trash2, kt[0:B, 0, 0:D]
                            )
                            nc.vector.tensor_copy(
                                trash2, vt[0:B, 0, 0:D]
                            )
                            continue
                        sc = scp.tile([128, H_LOC, ET], F32)
                        pr = prp.tile([128, H_LOC, ET], F32)
                        for h in range(H_LOC):
                            bh = h * B + b
                            for t in range(n_t):
                                # last tile may be partial: pre-fill the score
                                # column with -1e4 (exp -> 0) and only compute
                                # the valid [0:rem] partitions
                                p_cnt = 128 if t < n_t - 1 else rem
                                if p_cnt < 128:
                                    nc.vector.memset(
                                        sc[:, h, t : t + 1], -10000.0
                                    )
                                trash = trp.tile([128, D], F32)
                                nc.vector.tensor_mul(
                                    trash[0:p_cnt, :],
                                    kt[0:p_cnt, t, h * D : (h + 1) * D],
                                    qb_sb[0:p_cnt, h * D : (h + 1) * D],
                                )
                                nc.vector.reduce_sum(
                                    out=sc[0:p_cnt, h, t : t + 1],
                                    in_=trash[0:p_cnt, :],
                                    axis=mybir.AxisListType.X,
                                )
                            if stage < 6:
                                continue
                            nc.scalar.activation(
                                pr[:, h, 0:n_t],
                                sc[:, h, 0:n_t],
                                EXP,
                                accum_out=sums_sb[:, bh : bh + 1],
                            )
                            if stage < 7:
                                continue
                            for t in range(n_t):
                                nc.tensor.matmul(
                                    attnT_ps[:, bh : bh + 1],
                                    vt[:, t, h * D : (h + 1) * D],
                                    pr[:, h, t : t + 1],
                                    start=(t == 0),
                                    stop=False,
                                    skip_group_check=True,
                                )
                            # fold in the new token's v, weighted by e_new
                            nc.tensor.matmul(
                                attnT_ps[:, bh : bh + 1],
                                v_new_sb[:, h * D : (h + 1) * D],
                                diag_sb[:, h, b : b + 1],
                                start=False,
                                stop=True,
                                skip_group_check=True,
                            )

                # ---------------- phase 3: normalize + out-projection -------
                with tc.tile_pool(name="ph3ps", bufs=1, space="PSUM") as ph3ps:
                    with tc.tile_pool(name="outps", bufs=1, space="PSUM") as outps:
                        if stage >= 99:
                            tot_ps = ph3ps.tile([1, H_LOC * B], F32, tag="tot")
                            nc.tensor.matmul(
                                tot_ps, ones_128, sums_sb, start=True, stop=False,
                                skip_group_check=True,
                            )
                            for h in range(H_LOC):
                                # adds e_new[b,h] into column h*B+b
                                nc.tensor.matmul(
                                    tot_ps[:, h * B : (h + 1) * B],
                                    ones_32,
                                    diag_sb[:, h, :],
                                    start=False,
                                    stop=(h == H_LOC - 1),
                                    skip_group_check=True,
                                )
                            nc.vector.reciprocal(recip_sb, tot_ps)
                            R_ps = ph3ps.tile([128, H_LOC * B], F32, tag="R")
                            nc.tensor.matmul(
                                R_ps, ones_1x128, recip_sb, start=True, stop=True
                            )
                            nc.vector.tensor_copy(R_sb, R_ps)
                            # normalize during the PSUM->SBUF move
                            nc.vector.tensor_mul(attn_sb, attnT_ps, R_sb)
                            out_ps = outps.tile([B, E], F32)
                            for h in range(H_LOC):
                                for j in range(4):
                                    nc.tensor.matmul(
                                        out_ps[:, j * 512 : (j + 1) * 512],
                                        attn_sb[:, h * B : (h + 1) * B],
                                        wout_sb[:, h, j * 512 : (j + 1) * 512],
                                        start=(h == 0),
                                        stop=(h == H_LOC - 1),
                                    )
                            nc.vector.tensor_copy(out_sb, out_ps)
                        else:
                            # bisect mode: dump q (and whatever ran) only
                            nc.vector.memset(out_sb, 0.0)
                            nc.vector.tensor_copy(out_sb[:, 0:CLOC], q_sb)
                        nc.sync.dma_start(out=out_d[:], in_=out_sb)
    nc.compile()
    return nc


def kernel(x, k_cache, v_cache, W_in, b_in, W_out, b_out, input_pos):
    global LAST_RESULT
    x = np.asarray(x)
    k_cache = np.asarray(k_cache)
    v_cache = np.asarray(v_cache)
    W_in = np.asarray(W_in, dtype=np.float32)
    b_in = np.asarray(b_in, dtype=np.float32)
    W_out = np.asarray(W_out, dtype=np.float32)
    b_out = np.asarray(b_out, dtype=np.float32)
    pos = np.asarray(input_pos).astype(np.int64)

    n_ts = []
    rems = []
    for b in range(B):
        s_old = int(pos[b]) - 1  # tokens already in the cache
        n_t = (s_old + ST - 1) // ST
        n_ts.append(n_t)
        rems.append(s_old - (n_t - 1) * ST if n_t > 0 else 0)
    key = (tuple(n_ts), tuple(rems), STAGE)
    if key not in _build_cache:
        _build_cache[key] = _build(key[0], key[1], STAGE)
    nc = _build_cache[key]

    x2 = np.ascontiguousarray(x.reshape(B, E), dtype=np.float32)
    in_maps = []
    for i in range(N_CORES):
        c0 = i * CLOC
        win_i = np.ascontiguousarray(
            np.concatenate(
                [
                    W_in[:, c0 : c0 + CLOC],
                    W_in[:, E + c0 : E + c0 + CLOC],
                    W_in[:, 2 * E + c0 : 2 * E + c0 + CLOC],
                ],
                axis=1,
            )
        )
        bin_i = np.ascontiguousarray(
            np.concatenate(
                [
                    b_in[c0 : c0 + CLOC],
                    b_in[E + c0 : E + c0 + CLOC],
                    b_in[2 * E + c0 : 2 * E + c0 + CLOC],
                ]
            )[None, :]
        )
        wout_i = np.ascontiguousarray(W_out[c0 : c0 + CLOC, :])
        h0 = i * H_LOC
        kc_i = np.ascontiguousarray(
            k_cache[:, :, h0 : h0 + H_LOC, :], dtype=np.float32
        ).reshape(B, S_MAX, CLOC)
        vc_i = np.ascontiguousarray(
            v_cache[:, :, h0 : h0 + H_LOC, :], dtype=np.float32
        ).reshape(B, S_MAX, CLOC)
        in_maps.append(
            {"x": x2, "win": win_i, "bin": bin_i, "wout": wout_i, "kc": kc_i, "vc": vc_i}
        )

    res = run_bass_kernel_spmd(nc, in_maps, core_ids=list(range(N_CORES)))
    LAST_RESULT = res
    out = np.zeros((B, E), dtype=np.float64)
    for r in res.results:
        out += r["out"].astype(np.float64)
    out += b_out.astype(np.float64)
    return out.astype(np.float32).reshape(B, 1, E)



# revision 15
# speedup vs baseline: 2.1416x; 2.1416x over previous
"""Decode-step KV-cache attention kernel for 8 Trainium2 NeuronCores.

Tensor-parallel over heads (2 heads per core, all 32 batch rows on every
core); per-core differences live in host-sliced inputs.  All bulk data is
bf16 (tolerance is 2e-2; bf16 keeps norm-relative error ~1e-3), which
halves HBM traffic — the binding constraint for this memory-regime problem
— and runs the PE at 1 cycle/row instead of fp32's 4.

Per-core pipeline (rows host-sorted by sequence length, descending):
  1. QKV projection: x^T tiles (PE transposes) x bf16 W_in -> PSUM;
     q and k_new are transposed on the PE into [d, row] layout.
  2. Scores, per (row, head, tile): one matmul with the host-TRANSPOSED
     K tile [d, tokens] as the stationary weights and the q column as
     the single moving vector -> scores [128 tokens, 1] in PSUM.  The
     new token's k is copied into the K tile at column L-1 beforehand,
     so there is no separate new-token path.  Pad tokens in the last
     tile are masked to -1e4 with a tiny memset.
  3. Exp, per (row, head): one activation over [128, n_tiles] with
     accum_out collecting per-partition sums; a single [128,64]x[128,1]
     matmul + reciprocal then yields 1/denominator per (row, head).
  4. PV, per (row, tile): one matmul with the 2 probability columns as
     weights and the V tile [tokens, 256ch] streaming -> [2, 256] PSUM
     accumulated over tiles, copied to SBUF per row.
  5. Normalize, transpose, out-project with bf16 W_out; host sums the 8
     partial outputs.
"""

import math
import sys

import numpy as np
import ml_dtypes

sys.path.insert(0, "/opt/trn_rl_repo")

import concourse.bass as bass  # noqa: E402
import concourse.tile as tile  # noqa: E402
from concourse import bacc, mybir  # noqa: E402
from concourse.bass_utils import run_bass_kernel_spmd  # noqa: E402
from concourse.masks import make_identity  # noqa: E402

B, S_MAX, H, D = 32, 2048, 16, 128
E = H * D  # 2048
N_CORES = 8
H_LOC = H // N_CORES  # 2 heads per core
CLOC = H_LOC * D  # 256
ET = E // 128  # 16 contraction tiles for the in-projection

F32 = mybir.dt.float32
BF16 = mybir.dt.bfloat16
NPBF = ml_dtypes.bfloat16
EXP = mybir.ActivationFunctionType.Exp

_build_cache: dict = {}
LAST_RESULT = None  # last BassKernelResults, for test harness introspection


def _build(Ls: tuple, obs: tuple) -> bass.Bass:
    """Per-core Bass program. Ls = sorted (descending) seq lengths;
    obs[j] = original batch index of sorted row j."""
    nts = [(l + 127) // 128 for l in Ls]
    k_off = []  # element offsets of each row's K block in the packed buffer
    v_off = []
    ko = vo = 0
    for nt in nts:
        k_off.append(ko)
        v_off.append(vo)
        ko += 128 * 2 * nt * 128
        vo += 128 * nt * 256

    nc = bacc.Bacc("TRN2")
    x_d = nc.dram_tensor("x", [B, E], BF16, kind="ExternalInput")
    win_d = nc.dram_tensor("win", [128, ET * 3 * CLOC], BF16, kind="ExternalInput")
    bin_d = nc.dram_tensor("bin", [1, 3 * CLOC], BF16, kind="ExternalInput")
    wout_d = nc.dram_tensor("wout", [128, H_LOC * E], BF16, kind="ExternalInput")
    kc_d = nc.dram_tensor("kc", [ko], BF16, kind="ExternalInput")
    vc_d = nc.dram_tensor("vc", [vo], BF16, kind="ExternalInput")
    corr_d = nc.dram_tensor("corr", [H_LOC, B], F32, kind="ExternalInput")
    out_d = nc.dram_tensor("out", [B, E], F32, kind="ExternalOutput")

    kc_base = kc_d[:]
    vc_base = vc_d[:]
    inv_sqrt_d = 1.0 / math.sqrt(D)

    with tile.TileContext(nc) as tc:
        with tc.tile_pool(name="const", bufs=1) as const:
            I64 = const.tile([64, 64], BF16)
            make_identity(nc, I64)
            I32 = I64[0:32, 0:32]
            ones_1x32 = const.tile([1, 32], BF16)
            nc.vector.memset(ones_1x32, 1.0)
            ones_128 = const.tile([128, 1], F32)
            nc.vector.memset(ones_128, 1.0)

            win_sb = const.tile([128, ET, 3 * CLOC], BF16)
            nc.sync.dma_start(out=win_sb, in_=win_d[:])
            bin_sb = const.tile([1, 3 * CLOC], BF16)
            nc.sync.dma_start(out=bin_sb, in_=bin_d[:])
            x_sb = const.tile([B, E], BF16)
            nc.sync.dma_start(out=x_sb, in_=x_d[:])
            wout_sb = const.tile([128, H_LOC, E], BF16)
            nc.scalar.dma_start(out=wout_sb, in_=wout_d[:])

            corr_sb = const.tile([H_LOC, B], F32)
            nc.sync.dma_start(out=corr_sb, in_=corr_d[:])

            xT_sb = const.tile([128, ET, B], BF16)
            q_sb = const.tile([B, CLOC], BF16)
            k_new_sb = const.tile([B, CLOC], BF16)
            v_new_sb = const.tile([B, CLOC], BF16)
            qT_sb = const.tile([128, H_LOC, B], BF16)
            k_newT_sb = const.tile([128, H_LOC, B], BF16)
            sums_sb = const.tile([128, 2 * B], F32)
            aT_sb = const.tile([128, H_LOC, 2 * B], BF16)
            out_sb = const.tile([B, E], F32)

            # ---------------- phase 1: fused QKV projection ----------------
            with tc.tile_pool(name="ph1ps", bufs=2, space="PSUM") as ph1ps:
                with tc.tile_pool(name="qkvps", bufs=1, space="PSUM") as qkvps:
                    for t in range(ET):
                        xt_ps = ph1ps.tile([128, B], BF16)
                        nc.tensor.transpose(
                            xt_ps, x_sb[:, t * 128 : (t + 1) * 128], I32
                        )
                        nc.vector.tensor_copy(xT_sb[:, t, :], xt_ps)
                    qkv_ps = qkvps.tile([B, 3 * CLOC], F32)
                    for c0, c1 in ((0, 512), (512, 768)):
                        nc.tensor.matmul(
                            qkv_ps[:, c0:c1],
                            ones_1x32,
                            bin_sb[:, c0:c1],
                            start=True,
                            stop=False,
                        )
                        for t in range(ET):
                            nc.tensor.matmul(
                                qkv_ps[:, c0:c1],
                                xT_sb[:, t, :],
                                win_sb[:, t, c0:c1],
                                start=False,
                                stop=(t == ET - 1),
                            )
                    nc.vector.tensor_copy(q_sb, qkv_ps[:, 0:CLOC])
                    nc.vector.tensor_copy(k_new_sb, qkv_ps[:, CLOC : 2 * CLOC])
                    nc.vector.tensor_copy(v_new_sb, qkv_ps[:, 2 * CLOC : 3 * CLOC])
                    for h in range(H_LOC):
                        qt_ps = ph1ps.tile([128, B], BF16, tag="qt")
                        nc.tensor.transpose(
                            qt_ps, q_sb[:, h * D : (h + 1) * D], I32
                        )
                        nc.vector.tensor_copy(qT_sb[:, h, :], qt_ps)
                        kt_ps = ph1ps.tile([128, B], BF16, tag="qt")
                        nc.tensor.transpose(
                            kt_ps, k_new_sb[:, h * D : (h + 1) * D], I32
                        )
                        nc.vector.tensor_copy(k_newT_sb[:, h, :], kt_ps)

            # ---------------- phase 2: scores -> exp -> PV ------------------
            with tc.tile_pool(name="scps", bufs=2, space="PSUM") as scps, \
                 tc.tile_pool(name="avps", bufs=2, space="PSUM") as avps, \
                 tc.tile_pool(name="dnp", bufs=2, space="PSUM") as dnp, \
                 tc.tile_pool(name="tps", bufs=2, space="PSUM") as tps, \
                 tc.tile_pool(name="ktp", bufs=4) as ktp, \
                 tc.tile_pool(name="vtp", bufs=6) as vtp, \
                 tc.tile_pool(name="prp", bufs=3) as prp, \
                 tc.tile_pool(name="arp", bufs=2) as arp:
                vts = [None] * B
                prs = [None] * B

                def emit_scores(j):
                    L = Ls[j]
                    nt = nts[j]
                    sp = nt * 128
                    kt = ktp.tile([128, H_LOC, S_MAX], BF16)
                    nc.sync.dma_start(
                        out=kt[:, :, :sp],
                        in_=bass.AP(
                            tensor=kc_base.tensor,
                            offset=k_off[j],
                            ap=[[2 * sp, 128], [sp, 2], [1, sp]],
                        ),
                    )
                    vt = vtp.tile([128, ET, CLOC], BF16)
                    vts[j] = vt
                    nc.gpsimd.dma_start(
                        out=vt[:, :nt, :],
                        in_=bass.AP(
                            tensor=vc_base.tensor,
                            offset=v_off[j],
                            ap=[[nt * 256, 128], [256, nt], [1, 256]],
                        ),
                    )
                    # fold the new token in at column L-1
                    col = L - 1
                    ob = obs[j]
                    for h in range(H_LOC):
                        nc.vector.tensor_copy(
                            kt[:, h, col : col + 1],
                            k_newT_sb[:, h, ob : ob + 1],
                        )
                    # cross-partition row insert must go through DMA
                    p_new, t_new = col % 128, col // 128
                    nc.gpsimd.dma_start(
                        out=vt[p_new : p_new + 1, t_new, :],
                        in_=v_new_sb[ob : ob + 1, :],
                    )
                    sc = scps.tile([128, H_LOC, ET], F32)
                    pr = prp.tile([128, H_LOC, ET], BF16)
                    prs[j] = pr
                    for h in range(H_LOC):
                        for t in range(nt):
                            nc.tensor.matmul(
                                sc[:, h, t : t + 1],
                                kt[:, h, t * 128 : (t + 1) * 128],
                                qT_sb[:, h, ob : ob + 1],
                                start=True,
                                stop=True,
                                skip_group_check=True,
                            )
                    for h in range(H_LOC):
                        nc.scalar.activation(
                            pr[:, h, 0:nt],
                            sc[:, h, 0:nt],
                            EXP,
                            scale=inv_sqrt_d,
                            accum_out=sums_sb[:, 2 * j + h : 2 * j + h + 1],
                        )

                def emit_pv(j):
                    nt = nts[j]
                    vt = vts[j]
                    pr = prs[j]
                    av = avps.tile([H_LOC, CLOC], F32)
                    for t in range(nt):
                        nc.tensor.matmul(
                            av,
                            pr[:, :, t],
                            vt[:, t, :],
                            start=(t == 0),
                            stop=(t == nt - 1),
                            skip_group_check=True,
                        )
                    # denominator for this row: column sums minus pad count
                    den_ps = dnp.tile([H_LOC, 1], F32)
                    nc.tensor.matmul(
                        den_ps,
                        sums_sb[:, 2 * j : 2 * j + 2],
                        ones_128,
                        start=True,
                        stop=True,
                        skip_group_check=True,
                    )
                    ar = arp.tile([H_LOC, CLOC], BF16)
                    den2 = arp.tile([H_LOC, 2], F32, tag="dn")
                    nc.vector.tensor_sub(
                        den2[:, 0:1], den_ps, corr_sb[:, j : j + 1]
                    )
                    nc.vector.reciprocal(den2[:, 1:2], den2[:, 0:1])
                    nc.vector.tensor_scalar_mul(ar, av, den2[:, 1:2])
                    for h in range(H_LOC):
                        at_ps = tps.tile([128, H_LOC], BF16)
                        nc.tensor.transpose(
                            at_ps, ar[:, h * D : (h + 1) * D], I64[0:2, 0:2]
                        )
                        nc.vector.tensor_copy(
                            aT_sb[:, h, 2 * j : 2 * j + 2], at_ps
                        )

                for j in range(B):
                    emit_scores(j)
                    if j > 0:
                        emit_pv(j - 1)
                emit_pv(B - 1)

            # ---------------- phase 3: out-projection -----------------------
            with tc.tile_pool(name="outps", bufs=1, space="PSUM") as outps:
                    out_ps = outps.tile([B, E], F32)
                    for j4 in range(4):
                        for h in range(H_LOC):
                            base = aT_sb[:, h, :]
                            lhsT = bass.AP(
                                tensor=base.tensor,
                                offset=base.offset + h,
                                ap=[base.ap[0], [2, B]],
                            )
                            nc.tensor.matmul(
                                out_ps[:, j4 * 512 : (j4 + 1) * 512],
                                lhsT,
                                wout_sb[:, h, j4 * 512 : (j4 + 1) * 512],
                                start=(h == 0),
                                stop=(h == H_LOC - 1),
                            )
                    nc.vector.tensor_copy(out_sb, out_ps)
                    nc.scalar.dma_start(out=out_d[:], in_=out_sb)
    nc.compile()
    return nc


def kernel(x, k_cache, v_cache, W_in, b_in, W_out, b_out, input_pos):
    global LAST_RESULT
    x = np.asarray(x)
    k_cache = np.asarray(k_cache)
    v_cache = np.asarray(v_cache)
    W_in = np.asarray(W_in, dtype=np.float32)
    b_in = np.asarray(b_in, dtype=np.float32)
    W_out = np.asarray(W_out, dtype=np.float32)
    b_out = np.asarray(b_out, dtype=np.float32)
    pos = np.asarray(input_pos).astype(np.int64)

    order = sorted(range(B), key=lambda b: -int(pos[b]))
    Ls = tuple(int(pos[b]) for b in order)
    nts = [(l + 127) // 128 for l in Ls]

    corr = np.zeros((H_LOC, B), dtype=np.float32)
    for j in range(B):
        corr[:, j] = nts[j] * 128 - Ls[j]

    key = (Ls, tuple(order))
    if key not in _build_cache:
        _build_cache[key] = _build(Ls, tuple(order))
    nc = _build_cache[key]

    x2 = np.ascontiguousarray(x.reshape(B, E)).astype(NPBF)
    kc_bf = k_cache.astype(NPBF)
    vc_bf = v_cache.astype(NPBF)

    in_maps = []
    for i in range(N_CORES):
        c0 = i * CLOC
        win_i = np.concatenate(
            [
                W_in[:, c0 : c0 + CLOC],
                W_in[:, E + c0 : E + c0 + CLOC],
                W_in[:, 2 * E + c0 : 2 * E + c0 + CLOC],
            ],
            axis=1,
        )  # [2048, 768]
        win_i = np.ascontiguousarray(
            win_i.reshape(ET, 128, 3 * CLOC).transpose(1, 0, 2).reshape(128, -1)
        ).astype(NPBF)
        bin_i = np.concatenate(
            [
                b_in[c0 : c0 + CLOC],
                b_in[E + c0 : E + c0 + CLOC],
                b_in[2 * E + c0 : 2 * E + c0 + CLOC],
            ]
        )[None, :].astype(NPBF)
        wout_i = np.ascontiguousarray(
            W_out[c0 : c0 + CLOC, :].reshape(H_LOC, 128, E)
            .transpose(1, 0, 2)
            .reshape(128, -1)
        ).astype(NPBF)
        h0 = i * H_LOC
        k_h = kc_bf[:, :, h0 : h0 + H_LOC, :]  # [B, S, 2, 128]
        v_h = vc_bf[:, :, h0 : h0 + H_LOC, :]
        k_blocks = []
        v_blocks = []
        for j in range(B):
            ob = order[j]
            L = Ls[j]
            nt = nts[j]
            sp = nt * 128
            kb = np.zeros((128, H_LOC, sp), dtype=NPBF)
            if L > 1:
                kb[:, :, : L - 1] = k_h[ob, : L - 1].transpose(2, 1, 0)
            k_blocks.append(kb.ravel())
            vb = np.zeros((sp, CLOC), dtype=NPBF)
            if L > 1:
                vb[: L - 1] = v_h[ob, : L - 1].reshape(L - 1, CLOC)
            v_blocks.append(
                vb.reshape(nt, 128, CLOC).transpose(1, 0, 2).ravel()
            )
        kc_i = np.concatenate(k_blocks)
        vc_i = np.concatenate(v_blocks)
        in_maps.append(
            {
                "x": x2,
                "win": win_i,
                "bin": bin_i,
                "wout": wout_i,
                "kc": kc_i,
                "vc": vc_i,
                "corr": corr,
            }
        )

    res = run_bass_kernel_spmd(nc, in_maps, core_ids=list(range(N_CORES)))
    LAST_RESULT = res
    acc = np.zeros((B, E), dtype=np.float64)
    for r in res.results:
        acc += r["out"].astype(np.float64)
    acc += b_out.astype(np.float64)
    out = np.zeros((B, E), dtype=np.float32)
    out[np.array(order)] = acc.astype(np.float32)
    return out.reshape(B, 1, E)


# revision 21
# speedup vs baseline: 2.1879x; 1.0216x over previous
"""Decode-step KV-cache attention kernel for 8 Trainium2 NeuronCores.

Tensor-parallel over heads (2 heads per core, all 32 batch rows on every
core); per-core differences live in host-sliced inputs.  All bulk data is
bf16 (tolerance is 2e-2; bf16 keeps norm-relative error ~1e-3), which
halves HBM traffic — the binding constraint for this memory-regime problem
— and runs the PE at 1 cycle/row instead of fp32's 4.

Per-core pipeline (rows host-sorted by sequence length, descending):
  1. QKV projection: x^T tiles (PE transposes) x bf16 W_in -> PSUM;
     q and k_new are transposed on the PE into [d, row] layout.
  2. Scores, per (row, head, tile): one matmul with the host-TRANSPOSED
     K tile [d, tokens] as the stationary weights and the q column as
     the single moving vector -> scores [128 tokens, 1] in PSUM.  The
     new token's k is copied into the K tile at column L-1 beforehand,
     so there is no separate new-token path.  Pad tokens in the last
     tile are masked to -1e4 with a tiny memset.
  3. Exp, per (row, head): one activation over [128, n_tiles] with
     accum_out collecting per-partition sums; a single [128,64]x[128,1]
     matmul + reciprocal then yields 1/denominator per (row, head).
  4. PV, per (row, tile): one matmul with the 2 probability columns as
     weights and the V tile [tokens, 256ch] streaming -> [2, 256] PSUM
     accumulated over tiles, copied to SBUF per row.
  5. Normalize, transpose, out-project with bf16 W_out; host sums the 8
     partial outputs.
"""

import math
import sys

import numpy as np
import ml_dtypes

sys.path.insert(0, "/opt/trn_rl_repo")

import concourse.bass as bass  # noqa: E402
import concourse.tile as tile  # noqa: E402
from concourse import bacc, mybir  # noqa: E402
from concourse.bass_utils import run_bass_kernel_spmd  # noqa: E402
from concourse.masks import make_identity  # noqa: E402

B, S_MAX, H, D = 32, 2048, 16, 128
E = H * D  # 2048
N_CORES = 8
H_LOC = H // N_CORES  # 2 heads per core
CLOC = H_LOC * D  # 256
ET = E // 128  # 16 contraction tiles for the in-projection

F32 = mybir.dt.float32
BF16 = mybir.dt.bfloat16
NPBF = ml_dtypes.bfloat16
EXP = mybir.ActivationFunctionType.Exp

_build_cache: dict = {}
LAST_RESULT = None  # last BassKernelResults, for test harness introspection


def _build(Ls: tuple, obs: tuple) -> bass.Bass:
    """Per-core Bass program. Ls = sorted (descending) seq lengths;
    obs[j] = original batch index of sorted row j."""
    nts = [(l + 127) // 128 for l in Ls]
    k_off = []  # element offsets of each row's K block in the packed buffer
    v_off = []
    ko = vo = 0
    for nt in nts:
        k_off.append(ko)
        v_off.append(vo)
        ko += 128 * 2 * nt * 128
        vo += 128 * nt * 256

    nc = bacc.Bacc("TRN2")
    x_d = nc.dram_tensor("x", [B, E], BF16, kind="ExternalInput")
    win_d = nc.dram_tensor("win", [128, ET * 3 * CLOC], BF16, kind="ExternalInput")
    bin_d = nc.dram_tensor("bin", [1, 3 * CLOC], BF16, kind="ExternalInput")
    wout_d = nc.dram_tensor("wout", [128, H_LOC * E], BF16, kind="ExternalInput")
    kc_d = nc.dram_tensor("kc", [ko], BF16, kind="ExternalInput")
    vc_d = nc.dram_tensor("vc", [vo], BF16, kind="ExternalInput")
    corr_d = nc.dram_tensor("corr", [H_LOC, B], F32, kind="ExternalInput")
    out_d = nc.dram_tensor("out", [B, E], F32, kind="ExternalOutput")

    kc_base = kc_d[:]
    vc_base = vc_d[:]
    inv_sqrt_d = 1.0 / math.sqrt(D)

    with tile.TileContext(nc) as tc:
        with tc.tile_pool(name="const", bufs=1) as const:
            I64 = const.tile([64, 64], BF16)
            make_identity(nc, I64)
            I32 = I64[0:32, 0:32]
            ones_1x32 = const.tile([1, 32], BF16)
            nc.vector.memset(ones_1x32, 1.0)
            ones_128 = const.tile([128, 1], F32)
            nc.vector.memset(ones_128, 1.0)

            x_sb = const.tile([B, E], BF16)
            nc.scalar.dma_start(out=x_sb, in_=x_d[:])
            win_sb = const.tile([128, ET, 3 * CLOC], BF16)
            nc.scalar.dma_start(out=win_sb, in_=win_d[:])
            bin_sb = const.tile([1, 3 * CLOC], BF16)
            nc.scalar.dma_start(out=bin_sb, in_=bin_d[:])
            wout_sb = const.tile([128, H_LOC, E], BF16)
            nc.scalar.dma_start(out=wout_sb, in_=wout_d[:])

            corr_sb = const.tile([H_LOC, B], F32)
            nc.scalar.dma_start(out=corr_sb, in_=corr_d[:])

            xT_sb = const.tile([128, ET, B], BF16)
            q_sb = const.tile([B, CLOC], BF16)
            k_new_sb = const.tile([B, CLOC], BF16)
            v_new_sb = const.tile([B, CLOC], BF16)
            qT_sb = const.tile([128, H_LOC, B], BF16)
            k_newT_sb = const.tile([128, H_LOC, B], BF16)
            sums_sb = const.tile([128, 2 * B], F32)
            aT_sb = const.tile([128, H_LOC, 2 * B], BF16)
            out_sb = const.tile([B, E], F32)

            # ---------------- phase 1: fused QKV projection ----------------
            with tc.tile_pool(name="ph1ps", bufs=2, space="PSUM") as ph1ps:
                with tc.tile_pool(name="qkvps", bufs=1, space="PSUM") as qkvps:
                    for t in range(ET):
                        xt_ps = ph1ps.tile([128, B], BF16)
                        nc.tensor.transpose(
                            xt_ps, x_sb[:, t * 128 : (t + 1) * 128], I32
                        )
                        nc.vector.tensor_copy(xT_sb[:, t, :], xt_ps)
                    qkv_ps = qkvps.tile([B, 3 * CLOC], F32)
                    for c0, c1 in ((0, 512), (512, 768)):
                        nc.tensor.matmul(
                            qkv_ps[:, c0:c1],
                            ones_1x32,
                            bin_sb[:, c0:c1],
                            start=True,
                            stop=False,
                        )
                        for t in range(ET):
                            nc.tensor.matmul(
                                qkv_ps[:, c0:c1],
                                xT_sb[:, t, :],
                                win_sb[:, t, c0:c1],
                                start=False,
                                stop=(t == ET - 1),
                            )
                    nc.vector.tensor_copy(q_sb, qkv_ps[:, 0:CLOC])
                    nc.vector.tensor_copy(k_new_sb, qkv_ps[:, CLOC : 2 * CLOC])
                    nc.vector.tensor_copy(v_new_sb, qkv_ps[:, 2 * CLOC : 3 * CLOC])
                    for h in range(H_LOC):
                        qt_ps = ph1ps.tile([128, B], BF16, tag="qt")
                        nc.tensor.transpose(
                            qt_ps, q_sb[:, h * D : (h + 1) * D], I32
                        )
                        nc.vector.tensor_copy(qT_sb[:, h, :], qt_ps)
                        kt_ps = ph1ps.tile([128, B], BF16, tag="qt")
                        nc.tensor.transpose(
                            kt_ps, k_new_sb[:, h * D : (h + 1) * D], I32
                        )
                        nc.vector.tensor_copy(k_newT_sb[:, h, :], kt_ps)

            # ---------------- phase 2: scores -> exp -> PV ------------------
            with tc.tile_pool(name="scps", bufs=2, space="PSUM") as scps, \
                 tc.tile_pool(name="avps", bufs=2, space="PSUM") as avps, \
                 tc.tile_pool(name="dnp", bufs=2, space="PSUM") as dnp, \
                 tc.tile_pool(name="tps", bufs=2, space="PSUM") as tps, \
                 tc.tile_pool(name="ktp", bufs=6) as ktp, \
                 tc.tile_pool(name="vtp", bufs=8) as vtp, \
                 tc.tile_pool(name="prp", bufs=4) as prp, \
                 tc.tile_pool(name="arp", bufs=3) as arp:
                vts = [None] * B
                prs = [None] * B

                def emit_scores(j):
                    L = Ls[j]
                    nt = nts[j]
                    sp = nt * 128
                    kt = ktp.tile([128, H_LOC, S_MAX], BF16)
                    k_eng = nc.sync
                    k_eng.dma_start(
                        out=kt[:, :, :sp],
                        in_=bass.AP(
                            tensor=kc_base.tensor,
                            offset=k_off[j],
                            ap=[[2 * sp, 128], [sp, 2], [1, sp]],
                        ),
                    )
                    vt = vtp.tile([128, ET, CLOC], BF16)
                    vts[j] = vt
                    nc.gpsimd.dma_start(
                        out=vt[:, :nt, :],
                        in_=bass.AP(
                            tensor=vc_base.tensor,
                            offset=v_off[j],
                            ap=[[nt * 256, 128], [256, nt], [1, 256]],
                        ),
                    )
                    # fold the new token in at column L-1
                    col = L - 1
                    ob = obs[j]
                    for h in range(H_LOC):
                        nc.vector.tensor_copy(
                            kt[:, h, col : col + 1],
                            k_newT_sb[:, h, ob : ob + 1],
                        )
                    # cross-partition row insert must go through DMA
                    p_new, t_new = col % 128, col // 128
                    nc.scalar.dma_start(
                        out=vt[p_new : p_new + 1, t_new, :],
                        in_=v_new_sb[ob : ob + 1, :],
                    )
                    sc = scps.tile([128, H_LOC, ET], F32)
                    pr = prp.tile([128, H_LOC, ET], BF16)
                    prs[j] = pr
                    for h in range(H_LOC):
                        for t in range(nt):
                            nc.tensor.matmul(
                                sc[:, h, t : t + 1],
                                kt[:, h, t * 128 : (t + 1) * 128],
                                qT_sb[:, h, ob : ob + 1],
                                start=True,
                                stop=True,
                                skip_group_check=True,
                            )
                    for h in range(H_LOC):
                        nc.scalar.activation(
                            pr[:, h, 0:nt],
                            sc[:, h, 0:nt],
                            EXP,
                            scale=inv_sqrt_d,
                            accum_out=sums_sb[:, 2 * j + h : 2 * j + h + 1],
                        )

                def emit_pv(j):
                    nt = nts[j]
                    vt = vts[j]
                    pr = prs[j]
                    av = avps.tile([H_LOC, CLOC], F32)
                    for t in range(nt):
                        nc.tensor.matmul(
                            av,
                            pr[:, :, t],
                            vt[:, t, :],
                            start=(t == 0),
                            stop=(t == nt - 1),
                            skip_group_check=True,
                        )
                    # denominator for this row: column sums minus pad count
                    den_ps = dnp.tile([H_LOC, 1], F32)
                    nc.tensor.matmul(
                        den_ps,
                        sums_sb[:, 2 * j : 2 * j + 2],
                        ones_128,
                        start=True,
                        stop=True,
                        skip_group_check=True,
                    )
                    ar = arp.tile([H_LOC, CLOC], BF16)
                    den2 = arp.tile([H_LOC, 2], F32, tag="dn")
                    nc.vector.tensor_sub(
                        den2[:, 0:1], den_ps, corr_sb[:, j : j + 1]
                    )
                    nc.vector.reciprocal(den2[:, 1:2], den2[:, 0:1])
                    nc.vector.tensor_scalar_mul(ar, av, den2[:, 1:2])
                    for h in range(H_LOC):
                        at_ps = tps.tile([128, H_LOC], BF16)
                        nc.tensor.transpose(
                            at_ps, ar[:, h * D : (h + 1) * D], I64[0:2, 0:2]
                        )
                        nc.vector.tensor_copy(
                            aT_sb[:, h, 2 * j : 2 * j + 2], at_ps
                        )

                for j in range(B):
                    emit_scores(j)
                    if j > 0:
                        emit_pv(j - 1)
                emit_pv(B - 1)

            # ---------------- phase 3: out-projection -----------------------
            with tc.tile_pool(name="outps", bufs=1, space="PSUM") as outps:
                    out_ps = outps.tile([B, E], F32)
                    for j4 in range(4):
                        for h in range(H_LOC):
                            base = aT_sb[:, h, :]
                            lhsT = bass.AP(
                                tensor=base.tensor,
                                offset=base.offset + h,
                                ap=[base.ap[0], [2, B]],
                            )
                            nc.tensor.matmul(
                                out_ps[:, j4 * 512 : (j4 + 1) * 512],
                                lhsT,
                                wout_sb[:, h, j4 * 512 : (j4 + 1) * 512],
                                start=(h == 0),
                                stop=(h == H_LOC - 1),
                            )
                    nc.vector.tensor_copy(out_sb, out_ps)
                    nc.scalar.dma_start(out=out_d[:], in_=out_sb)
    nc.compile()
    return nc


def kernel(x, k_cache, v_cache, W_in, b_in, W_out, b_out, input_pos):
    global LAST_RESULT
    x = np.asarray(x)
    k_cache = np.asarray(k_cache)
    v_cache = np.asarray(v_cache)
    W_in = np.asarray(W_in, dtype=np.float32)
    b_in = np.asarray(b_in, dtype=np.float32)
    W_out = np.asarray(W_out, dtype=np.float32)
    b_out = np.asarray(b_out, dtype=np.float32)
    pos = np.asarray(input_pos).astype(np.int64)

    order = sorted(range(B), key=lambda b: -int(pos[b]))
    Ls = tuple(int(pos[b]) for b in order)
    nts = [(l + 127) // 128 for l in Ls]

    corr = np.zeros((H_LOC, B), dtype=np.float32)
    for j in range(B):
        corr[:, j] = nts[j] * 128 - Ls[j]

    key = (Ls, tuple(order))
    if key not in _build_cache:
        _build_cache[key] = _build(Ls, tuple(order))
    nc = _build_cache[key]

    x2 = np.ascontiguousarray(x.reshape(B, E)).astype(NPBF)
    kc_bf = k_cache.astype(NPBF)
    vc_bf = v_cache.astype(NPBF)

    in_maps = []
    for i in range(N_CORES):
        c0 = i * CLOC
        win_i = np.concatenate(
            [
                W_in[:, c0 : c0 + CLOC],
                W_in[:, E + c0 : E + c0 + CLOC],
                W_in[:, 2 * E + c0 : 2 * E + c0 + CLOC],
            ],
            axis=1,
        )  # [2048, 768]
        win_i = np.ascontiguousarray(
            win_i.reshape(ET, 128, 3 * CLOC).transpose(1, 0, 2).reshape(128, -1)
        ).astype(NPBF)
        bin_i = np.concatenate(
            [
                b_in[c0 : c0 + CLOC],
                b_in[E + c0 : E + c0 + CLOC],
                b_in[2 * E + c0 : 2 * E + c0 + CLOC],
            ]
        )[None, :].astype(NPBF)
        wout_i = np.ascontiguousarray(
            W_out[c0 : c0 + CLOC, :].reshape(H_LOC, 128, E)
            .transpose(1, 0, 2)
            .reshape(128, -1)
        ).astype(NPBF)
        h0 = i * H_LOC
        k_h = kc_bf[:, :, h0 : h0 + H_LOC, :]  # [B, S, 2, 128]
        v_h = vc_bf[:, :, h0 : h0 + H_LOC, :]
        k_blocks = []
        v_blocks = []
        for j in range(B):
            ob = order[j]
            L = Ls[j]
            nt = nts[j]
            sp = nt * 128
            kb = np.zeros((128, H_LOC, sp), dtype=NPBF)
            if L > 1:
                kb[:, :, : L - 1] = k_h[ob, : L - 1].transpose(2, 1, 0)
            k_blocks.append(kb.ravel())
            vb = np.zeros((sp, CLOC), dtype=NPBF)
            if L > 1:
                vb[: L - 1] = v_h[ob, : L - 1].reshape(L - 1, CLOC)
            v_blocks.append(
                vb.reshape(nt, 128, CLOC).transpose(1, 0, 2).ravel()
            )
        kc_i = np.concatenate(k_blocks)
        vc_i = np.concatenate(v_blocks)
        in_maps.append(
            {
                "x": x2,
                "win": win_i,
                "bin": bin_i,
                "wout": wout_i,
                "kc": kc_i,
                "vc": vc_i,
                "corr": corr,
            }
        )

    res = run_bass_kernel_spmd(nc, in_maps, core_ids=list(range(N_CORES)))
    LAST_RESULT = res
    acc = np.zeros((B, E), dtype=np.float64)
    for r in res.results:
        acc += r["out"].astype(np.float64)
    acc += b_out.astype(np.float64)
    out = np.zeros((B, E), dtype=np.float32)
    out[np.array(order)] = acc.astype(np.float32)
    return out.reshape(B, 1, E)


# revision 26
# speedup vs baseline: 2.4092x; 1.1011x over previous
"""Decode-step KV-cache attention kernel for 8 Trainium2 NeuronCores.

Tensor-parallel over heads (2 heads per core, all 32 batch rows on every
core); per-core differences live in host-sliced inputs.  All bulk data is
bf16 (tolerance is 2e-2; bf16 keeps norm-relative error ~1e-3), which
halves HBM traffic — the binding constraint for this memory-regime problem
— and runs the PE at 1 cycle/row instead of fp32's 4.

Per-core pipeline (rows host-sorted by sequence length, descending):
  1. QKV projection: x^T tiles (PE transposes) x bf16 W_in -> PSUM;
     q and k_new are transposed on the PE into [d, row] layout.
  2. Scores, per (row, head, tile): one matmul with the host-TRANSPOSED
     K tile [d, tokens] as the stationary weights and the q column as
     the single moving vector -> scores [128 tokens, 1] in PSUM.  The
     new token's k is copied into the K tile at column L-1 beforehand,
     so there is no separate new-token path.  Pad tokens in the last
     tile are masked to -1e4 with a tiny memset.
  3. Exp, per (row, head): one activation over [128, n_tiles] with
     accum_out collecting per-partition sums; a single [128,64]x[128,1]
     matmul + reciprocal then yields 1/denominator per (row, head).
  4. PV, per (row, tile): one matmul with the 2 probability columns as
     weights and the V tile [tokens, 256ch] streaming -> [2, 256] PSUM
     accumulated over tiles, copied to SBUF per row.
  5. Normalize, transpose, out-project with bf16 W_out; host sums the 8
     partial outputs.
"""

import math
import sys

import numpy as np
import ml_dtypes

sys.path.insert(0, "/opt/trn_rl_repo")

import concourse.bass as bass  # noqa: E402
import concourse.tile as tile  # noqa: E402
from concourse import bacc, mybir  # noqa: E402
from concourse.bass_utils import run_bass_kernel_spmd  # noqa: E402
from concourse.masks import make_identity  # noqa: E402

B, S_MAX, H, D = 32, 2048, 16, 128
E = H * D  # 2048
N_CORES = 8
H_LOC = H // N_CORES  # 2 heads per core
CLOC = H_LOC * D  # 256
ET = E // 128  # 16 contraction tiles for the in-projection

F32 = mybir.dt.float32
BF16 = mybir.dt.bfloat16
NPBF = ml_dtypes.bfloat16
EXP = mybir.ActivationFunctionType.Exp

_build_cache: dict = {}
LAST_RESULT = None  # last BassKernelResults, for test harness introspection


def _build(Ls: tuple, obs: tuple) -> bass.Bass:
    """Per-core Bass program. Ls = sorted (descending) seq lengths;
    obs[j] = original batch index of sorted row j."""
    nts = [(l + 127) // 128 for l in Ls]
    k_off = []  # element offsets of each row's K block in the packed buffer
    v_off = []
    ko = vo = 0
    for nt in nts:
        k_off.append(ko)
        v_off.append(vo)
        ko += 128 * 2 * nt * 128
        vo += 128 * nt * 256

    nc = bacc.Bacc("TRN2")
    x_d = nc.dram_tensor("x", [B, E], BF16, kind="ExternalInput")
    win_d = nc.dram_tensor("win", [128, ET * 3 * CLOC], BF16, kind="ExternalInput")
    bin_d = nc.dram_tensor("bin", [1, 3 * CLOC], BF16, kind="ExternalInput")
    wout_d = nc.dram_tensor("wout", [128, H_LOC * E], BF16, kind="ExternalInput")
    kc_d = nc.dram_tensor("kc", [ko], BF16, kind="ExternalInput")
    vc_d = nc.dram_tensor("vc", [vo], BF16, kind="ExternalInput")
    corr_d = nc.dram_tensor("corr", [H_LOC, B], F32, kind="ExternalInput")
    out_d = nc.dram_tensor("out", [B, E], F32, kind="ExternalOutput")

    kc_base = kc_d[:]
    vc_base = vc_d[:]
    inv_sqrt_d = 1.0 / math.sqrt(D)

    with tile.TileContext(nc) as tc:
        with tc.tile_pool(name="const", bufs=1) as const:
            I64 = const.tile([64, 64], BF16)
            make_identity(nc, I64)
            I32 = I64[0:32, 0:32]
            ones_1x32 = const.tile([1, 32], BF16)
            nc.vector.memset(ones_1x32, 1.0)
            ones_128 = const.tile([128, 1], F32)
            nc.vector.memset(ones_128, 1.0)

            x_sb = const.tile([B, E], BF16)
            nc.scalar.dma_start(out=x_sb, in_=x_d[:])
            win_sb = const.tile([128, ET, 3 * CLOC], BF16)
            nc.scalar.dma_start(out=win_sb, in_=win_d[:])
            bin_sb = const.tile([1, 3 * CLOC], BF16)
            nc.scalar.dma_start(out=bin_sb, in_=bin_d[:])
            wout_sb = const.tile([128, H_LOC, E], BF16)
            nc.sync.dma_start(out=wout_sb, in_=wout_d[:])

            corr_sb = const.tile([H_LOC, B], F32)
            nc.sync.dma_start(out=corr_sb, in_=corr_d[:])

            xT_sb = const.tile([128, ET, B], BF16)
            q_sb = const.tile([B, CLOC], BF16)
            k_new_sb = const.tile([B, CLOC], BF16)
            v_new_sb = const.tile([B, CLOC], BF16)
            qT_sb = const.tile([128, H_LOC, B], BF16)
            k_newT_sb = const.tile([128, H_LOC, B], BF16)
            sums_sb = const.tile([128, 2 * B], F32)
            aT_sb = const.tile([128, H_LOC, 2 * B], BF16)
            out_sb = const.tile([B, E], F32)

            # ---------------- phase 1: fused QKV projection ----------------
            with tc.tile_pool(name="ph1ps", bufs=2, space="PSUM") as ph1ps:
                with tc.tile_pool(name="qkvps", bufs=1, space="PSUM") as qkvps:
                    for t in range(ET):
                        xt_ps = ph1ps.tile([128, B], BF16)
                        nc.tensor.transpose(
                            xt_ps, x_sb[:, t * 128 : (t + 1) * 128], I32
                        )
                        nc.vector.tensor_copy(xT_sb[:, t, :], xt_ps)
                    qkv_ps = qkvps.tile([B, 3 * CLOC], F32)
                    for c0, c1 in ((0, 512), (512, 768)):
                        nc.tensor.matmul(
                            qkv_ps[:, c0:c1],
                            ones_1x32,
                            bin_sb[:, c0:c1],
                            start=True,
                            stop=False,
                        )
                        for t in range(ET):
                            nc.tensor.matmul(
                                qkv_ps[:, c0:c1],
                                xT_sb[:, t, :],
                                win_sb[:, t, c0:c1],
                                start=False,
                                stop=(t == ET - 1),
                            )
                    nc.vector.tensor_copy(q_sb, qkv_ps[:, 0:CLOC])
                    nc.vector.tensor_copy(k_new_sb, qkv_ps[:, CLOC : 2 * CLOC])
                    nc.vector.tensor_copy(v_new_sb, qkv_ps[:, 2 * CLOC : 3 * CLOC])
                    for h in range(H_LOC):
                        qt_ps = ph1ps.tile([128, B], BF16, tag="qt")
                        nc.tensor.transpose(
                            qt_ps, q_sb[:, h * D : (h + 1) * D], I32
                        )
                        nc.vector.tensor_copy(qT_sb[:, h, :], qt_ps)
                        kt_ps = ph1ps.tile([128, B], BF16, tag="qt")
                        nc.tensor.transpose(
                            kt_ps, k_new_sb[:, h * D : (h + 1) * D], I32
                        )
                        nc.vector.tensor_copy(k_newT_sb[:, h, :], kt_ps)

            # ---------------- phase 2: scores -> exp -> PV ------------------
            with tc.tile_pool(name="scps", bufs=2, space="PSUM") as scps, \
                 tc.tile_pool(name="avps", bufs=2, space="PSUM") as avps, \
                 tc.tile_pool(name="dnp", bufs=2, space="PSUM") as dnp, \
                 tc.tile_pool(name="tps", bufs=2, space="PSUM") as tps, \
                 tc.tile_pool(name="ktp", bufs=8) as ktp, \
                 tc.tile_pool(name="vtp", bufs=8) as vtp, \
                 tc.tile_pool(name="prp", bufs=4) as prp, \
                 tc.tile_pool(name="arp", bufs=3) as arp:
                vts = [None] * B
                prs = [None] * B

                def emit_scores(j):
                    L = Ls[j]
                    nt = nts[j]
                    sp = nt * 128
                    kt = ktp.tile([128, H_LOC, S_MAX], BF16)
                    k_eng = nc.scalar
                    k_eng.dma_start(
                        out=kt[:, :, :sp],
                        in_=bass.AP(
                            tensor=kc_base.tensor,
                            offset=k_off[j],
                            ap=[[2 * sp, 128], [sp, 2], [1, sp]],
                        ),
                    )
                    vt = vtp.tile([128, ET, CLOC], BF16)
                    vts[j] = vt
                    nc.gpsimd.dma_start(
                        out=vt[:, :nt, :],
                        in_=bass.AP(
                            tensor=vc_base.tensor,
                            offset=v_off[j],
                            ap=[[nt * 256, 128], [256, nt], [1, 256]],
                        ),
                    )
                    # fold the new token in at column L-1
                    col = L - 1
                    ob = obs[j]
                    for h in range(H_LOC):
                        nc.vector.tensor_copy(
                            kt[:, h, col : col + 1],
                            k_newT_sb[:, h, ob : ob + 1],
                        )
                    # cross-partition row insert must go through DMA
                    p_new, t_new = col % 128, col // 128
                    nc.sync.dma_start(
                        out=vt[p_new : p_new + 1, t_new, :],
                        in_=v_new_sb[ob : ob + 1, :],
                    )
                    sc = scps.tile([128, H_LOC, ET], F32)
                    pr = prp.tile([128, H_LOC, ET], BF16)
                    prs[j] = pr
                    for h in range(H_LOC):
                        for t in range(nt):
                            nc.tensor.matmul(
                                sc[:, h, t : t + 1],
                                kt[:, h, t * 128 : (t + 1) * 128],
                                qT_sb[:, h, ob : ob + 1],
                                start=True,
                                stop=True,
                                skip_group_check=True,
                            )
                    for h in range(H_LOC):
                        nc.scalar.activation(
                            pr[:, h, 0:nt],
                            sc[:, h, 0:nt],
                            EXP,
                            scale=inv_sqrt_d,
                            accum_out=sums_sb[:, 2 * j + h : 2 * j + h + 1],
                        )

                def emit_pv(j):
                    nt = nts[j]
                    vt = vts[j]
                    pr = prs[j]
                    av = avps.tile([H_LOC, CLOC], F32)
                    for t in range(nt):
                        nc.tensor.matmul(
                            av,
                            pr[:, :, t],
                            vt[:, t, :],
                            start=(t == 0),
                            stop=(t == nt - 1),
                            skip_group_check=True,
                        )
                    # denominator for this row: column sums minus pad count
                    den_ps = dnp.tile([H_LOC, 1], F32)
                    nc.tensor.matmul(
                        den_ps,
                        sums_sb[:, 2 * j : 2 * j + 2],
                        ones_128,
                        start=True,
                        stop=True,
                        skip_group_check=True,
                    )
                    ar = arp.tile([H_LOC, CLOC], BF16)
                    den2 = arp.tile([H_LOC, 2], F32, tag="dn")
                    nc.vector.tensor_sub(
                        den2[:, 0:1], den_ps, corr_sb[:, j : j + 1]
                    )
                    nc.vector.reciprocal(den2[:, 1:2], den2[:, 0:1])
                    nc.vector.tensor_scalar_mul(ar, av, den2[:, 1:2])
                    for h in range(H_LOC):
                        at_ps = tps.tile([128, H_LOC], BF16)
                        nc.tensor.transpose(
                            at_ps, ar[:, h * D : (h + 1) * D], I64[0:2, 0:2]
                        )
                        nc.vector.tensor_copy(
                            aT_sb[:, h, 2 * j : 2 * j + 2], at_ps
                        )

                for j in range(B):
                    emit_scores(j)
                    if j > 0:
                        emit_pv(j - 1)
                emit_pv(B - 1)

            # ---------------- phase 3: out-projection -----------------------
            with tc.tile_pool(name="outps", bufs=1, space="PSUM") as outps:
                    out_ps = outps.tile([B, E], F32)
                    for j4 in range(4):
                        for h in range(H_LOC):
                            base = aT_sb[:, h, :]
                            lhsT = bass.AP(
                                tensor=base.tensor,
                                offset=base.offset + h,
                                ap=[base.ap[0], [2, B]],
                            )
                            nc.tensor.matmul(
                                out_ps[:, j4 * 512 : (j4 + 1) * 512],
                                lhsT,
                                wout_sb[:, h, j4 * 512 : (j4 + 1) * 512],
                                start=(h == 0),
                                stop=(h == H_LOC - 1),
                            )
                    nc.vector.tensor_copy(out_sb, out_ps)
                    nc.sync.dma_start(out=out_d[:], in_=out_sb)
    nc.compile()
    return nc


def kernel(x, k_cache, v_cache, W_in, b_in, W_out, b_out, input_pos):
    global LAST_RESULT
    x = np.asarray(x)
    k_cache = np.asarray(k_cache)
    v_cache = np.asarray(v_cache)
    W_in = np.asarray(W_in, dtype=np.float32)
    b_in = np.asarray(b_in, dtype=np.float32)
    W_out = np.asarray(W_out, dtype=np.float32)
    b_out = np.asarray(b_out, dtype=np.float32)
    pos = np.asarray(input_pos).astype(np.int64)

    order = sorted(range(B), key=lambda b: -int(pos[b]))
    Ls = tuple(int(pos[b]) for b in order)
    nts = [(l + 127) // 128 for l in Ls]

    corr = np.zeros((H_LOC, B), dtype=np.float32)
    for j in range(B):
        corr[:, j] = nts[j] * 128 - Ls[j]

    key = (Ls, tuple(order))
    if key not in _build_cache:
        _build_cache[key] = _build(Ls, tuple(order))
    nc = _build_cache[key]

    x2 = np.ascontiguousarray(x.reshape(B, E)).astype(NPBF)
    kc_bf = k_cache.astype(NPBF)
    vc_bf = v_cache.astype(NPBF)

    in_maps = []
    for i in range(N_CORES):
        c0 = i * CLOC
        win_i = np.concatenate(
            [
                W_in[:, c0 : c0 + CLOC],
                W_in[:, E + c0 : E + c0 + CLOC],
                W_in[:, 2 * E + c0 : 2 * E + c0 + CLOC],
            ],
            axis=1,
        )  # [2048, 768]
        win_i = np.ascontiguousarray(
            win_i.reshape(ET, 128, 3 * CLOC).transpose(1, 0, 2).reshape(128, -1)
        ).astype(NPBF)
        bin_i = np.concatenate(
            [
                b_in[c0 : c0 + CLOC],
                b_in[E + c0 : E + c0 + CLOC],
                b_in[2 * E + c0 : 2 * E + c0 + CLOC],
            ]
        )[None, :].astype(NPBF)
        wout_i = np.ascontiguousarray(
            W_out[c0 : c0 + CLOC, :].reshape(H_LOC, 128, E)
            .transpose(1, 0, 2)
            .reshape(128, -1)
        ).astype(NPBF)
        h0 = i * H_LOC
        k_h = kc_bf[:, :, h0 : h0 + H_LOC, :]  # [B, S, 2, 128]
        v_h = vc_bf[:, :, h0 : h0 + H_LOC, :]
        k_blocks = []
        v_blocks = []
        for j in range(B):
            ob = order[j]
            L = Ls[j]
            nt = nts[j]
            sp = nt * 128
            kb = np.zeros((128, H_LOC, sp), dtype=NPBF)
            if L > 1:
                kb[:, :, : L - 1] = k_h[ob, : L - 1].transpose(2, 1, 0)
            k_blocks.append(kb.ravel())
            vb = np.zeros((sp, CLOC), dtype=NPBF)
            if L > 1:
                vb[: L - 1] = v_h[ob, : L - 1].reshape(L - 1, CLOC)
            v_blocks.append(
                vb.reshape(nt, 128, CLOC).transpose(1, 0, 2).ravel()
            )
        kc_i = np.concatenate(k_blocks)
        vc_i = np.concatenate(v_blocks)
        in_maps.append(
            {
                "x": x2,
                "win": win_i,
                "bin": bin_i,
                "wout": wout_i,
                "kc": kc_i,
                "vc": vc_i,
                "corr": corr,
            }
        )

    res = run_bass_kernel_spmd(nc, in_maps, core_ids=list(range(N_CORES)))
    LAST_RESULT = res
    acc = np.zeros((B, E), dtype=np.float64)
    for r in res.results:
        acc += r["out"].astype(np.float64)
    acc += b_out.astype(np.float64)
    out = np.zeros((B, E), dtype=np.float32)
    out[np.array(order)] = acc.astype(np.float32)
    return out.reshape(B, 1, E)
